# revision 15
# baseline (speedup 1.0000x reference)
"""Trainium2 Bass kernel for nn_CopyStack (copy-mechanism vocab scatter).

Computes, for full inputs:
    enc   = tanh(encoder_outputs @ W_proj + b_proj)          [B,S,H]
    score = decoder_outputs @ enc^T + input_bias             [B,T,S]
    probs = softmax(score, axis=-1)                          [B,T,S]
    out[b,t,v] = sum_{s: inputs[b,s]==v} probs[b,t,s]        [B,T,V]

Sharding: 8 cores; core c handles batch b=c//2, target rows
t in [128*(c%2), 128*(c%2)+128). W_proj/b_proj replicated; E/D are
passed host-transposed ([H,S]/[H,TC]) and fp16-cast so matmul operands
load directly with no on-chip transposes (fp16 keeps an 11-bit
mantissa — measured end-to-end rel err ~4e-3 vs the 2e-2 gate).

Dataflow per core:
  - HWDGE zero-fills a [V, TC] bf16 staging buffer in DRAM (overlapped
    with compute).
  - encT = tanh(W^T E^T + b) via PE with fp16 operands (full-rate
    matmul + fast weight loads), k-outer accumulation into 4 PSUM
    banks so matmuls start as soon as the first chunks land.
  - softmax on ACT/DVE; duplicate token ids are pre-combined with a
    matmul against the S x S equality matrix C (every duplicate column
    carries its group sum, so colliding DMA writes are identical).
  - GPSIMD indirect DMA scatters bf16 rows of probs2^T into stg[V,TC].
  - 10 groups of 3200 v-rows are read back, PE-transposed (bf16,
    1 cyc/row), and streamed to the [TC, V] bf16 output; the host
    widens to f32 during the gather.
"""

import numpy as np

import concourse.bacc as bacc
import concourse.bass as bass
import concourse.tile as tile
from concourse import mybir
from concourse.bass import IndirectOffsetOnAxis
from concourse.bass_utils import run_bass_kernel_spmd
from concourse.masks import make_identity

F32 = mybir.dt.float32
F16 = mybir.dt.float16
BF16 = mybir.dt.bfloat16
I32 = mybir.dt.int32

B, S, T, H, V = 4, 512, 256, 1024, 32000
TC = 128             # T-chunk per core
N_CORES = 8

KH = H // 128        # 8 hidden chunks
KS = S // 128        # 4 source chunks
VG = 3200            # v-rows per readback group
NVB = VG // 128      # 25 v-blocks per group
NG = V // VG         # 10 groups

MM_DT = F16          # matmul operand dtype (host casts E/D/W)
STG_DT = BF16        # staging + output dtype (full f32 exponent range)
MM_NP = np.float16


def build_bass():
    nc = bacc.Bacc()

    eT_d = nc.dram_tensor("eT", [H, S], MM_DT, kind="ExternalInput")   # enc^T input
    dT_d = nc.dram_tensor("dT", [H, TC], MM_DT, kind="ExternalInput")  # dec chunk^T
    w = nc.dram_tensor("w", [H, H], MM_DT, kind="ExternalInput")       # W_proj
    ids = nc.dram_tensor("ids", [S], I32, kind="ExternalInput")        # inputs[b]
    sbias = nc.dram_tensor("sbias", [S], F32, kind="ExternalInput")    # input_bias[b]
    bproj = nc.dram_tensor("bproj", [H], F32, kind="ExternalInput")    # b_proj

    out = nc.dram_tensor("out", [TC, V], STG_DT, kind="ExternalOutput")

    with tile.TileContext(nc) as tc:
        with (
            tc.tile_pool(name="big", bufs=1) as big,
            tc.tile_pool(name="work", bufs=1) as work,
            tc.tile_pool(name="tpp", bufs=3) as tpp,
            tc.tile_pool(name="psum", bufs=2, space="PSUM") as psum,
            tc.tile_pool(name="psum16", bufs=2, space="PSUM") as psum16,
            tc.tile_pool(name="psumacc", bufs=4, space="PSUM") as psumacc,
            tc.tile_pool(name="dram", bufs=1, space="DRAM") as dram,
        ):
            stg = dram.tile([V, TC], STG_DT, tag="stg")
            stg_ap = stg[:, :]

            ident = work.tile([128, 128], F32, tag="ident")
            make_identity(nc, ident[:])
            ident16 = work.tile([128, 128], STG_DT, tag="ident16")
            nc.vector.tensor_copy(ident16[:], ident[:])

            # PE warm-up: sustained dummy work flips the HAM clock gate to
            # full rate (~3.4us of activity) before the real matmuls land.
            for wu in range(56):
                wpt = psum16.tile([128, 128], STG_DT, tag="tp16", name=f"wu{wu}")
                nc.tensor.transpose(
                    out=wpt[:], in_=ident16[:], identity=ident16[:])

            # ---- loads (no on-chip transposes: inputs come pre-transposed) ----
            w_t = []
            eT = []
            for k in range(KH):
                wt = big.tile([128, H], MM_DT, tag=f"w{k}")
                nc.sync.dma_start(wt[:], w[k * 128:(k + 1) * 128, :])
                w_t.append(wt)
                et = big.tile([128, S], MM_DT, tag=f"eT{k}")
                nc.sync.dma_start(et[:], eT_d[k * 128:(k + 1) * 128, :])
                eT.append(et)
            dT = []
            for k in range(KH):
                t_ = work.tile([128, TC], MM_DT, tag=f"dT{k}")
                nc.sync.dma_start(t_[:], dT_d[k * 128:(k + 1) * 128, :])
                dT.append(t_)

            ids_sb = work.tile([128, KS], I32, tag="ids")
            nc.sync.dma_start(ids_sb[:], ids[:].rearrange("(c p) -> p c", p=128))
            sbias_sb = work.tile([128, KS], F32, tag="sbias")
            nc.sync.dma_start(sbias_sb[:], sbias[:].rearrange("(c p) -> p c", p=128))
            bproj_sb = work.tile([128, KH], F32, tag="bproj")
            nc.sync.dma_start(bproj_sb[:], bproj[:].rearrange("(c p) -> p c", p=128))

            # ---- zero-fill the [V, TC] scatter buffer (big HWDGE DMAs) ----
            zt = big.tile([128, 4000], STG_DT, tag="zt")
            nc.vector.memset(zt[:], 0.0)
            stg_flat = stg_ap.rearrange("v t -> (v t)").rearrange(
                "(p f) -> p f", p=128)          # [128, 32000] flat view
            for j in range(8):
                nc.scalar.dma_start(stg_flat[:, j * 4000:(j + 1) * 4000], zt[:])

            # ---- encT[m] = tanh(W^T @ E^T + b) -> [128(h'), S] ----
            # k-outer so matmuls start when the first w/eT chunks land;
            # 4 PSUM banks per pass, 2 passes over m.
            encT = [None] * KH
            for half in range(2):
                ms = range(half * 4, half * 4 + 4)
                pms = {
                    m: psumacc.tile([128, S], F32, tag="mm", name=f"pm_{m}")
                    for m in ms
                }
                for k in range(KH):
                    for m in ms:
                        nc.tensor.matmul(
                            pms[m][:], lhsT=w_t[k][:, m * 128:(m + 1) * 128],
                            rhs=eT[k][:],
                            start=(k == 0), stop=(k == KH - 1),
                        )
                for m in ms:
                    et = big.tile([128, S], MM_DT, tag=f"encT{m}")
                    nc.scalar.activation(
                        et[:], pms[m][:], mybir.ActivationFunctionType.Tanh,
                        bias=bproj_sb[:, m:m + 1], scale=1.0,
                    )
                    encT[m] = et

            # ---- scores[t,s] = sum_h' dT[h',t] * encT[h',s] ----
            ps = psumacc.tile([128, S], F32, tag="mm")
            for k in range(KH):
                nc.tensor.matmul(
                    ps[:], lhsT=dT[k][:], rhs=encT[k][:],
                    start=(k == 0), stop=(k == KH - 1),
                )

            # bias row: [128, S] replicated over partitions, via PE transpose
            bias_row = work.tile([128, S], F32, tag="bias_row")
            for c in range(KS):
                pt = psum.tile([128, 128], F32, tag="tp")
                nc.tensor.transpose(
                    out=pt[:],
                    in_=sbias_sb[:, c:c + 1].to_broadcast([128, 128]),
                    identity=ident[:],
                )
                nc.vector.tensor_copy(bias_row[:, c * 128:(c + 1) * 128], pt[:])

            scoresb = work.tile([128, S], F32, tag="scoresb")
            nc.vector.tensor_tensor(
                out=scoresb[:], in0=ps[:], in1=bias_row[:], op=mybir.AluOpType.add,
            )

            # ---- softmax over s ----
            rmax = work.tile([128, 1], F32, tag="rmax")
            nc.vector.reduce_max(rmax[:], scoresb[:], axis=mybir.AxisListType.X)
            nrmax = work.tile([128, 1], F32, tag="nrmax")
            nc.vector.tensor_scalar_mul(nrmax[:], rmax[:], -1.0)
            ex = work.tile([128, S], F32, tag="ex")
            rsum = work.tile([128, 1], F32, tag="rsum")
            nc.scalar.activation(
                ex[:], scoresb[:], mybir.ActivationFunctionType.Exp,
                bias=nrmax[:, :1], scale=1.0, accum_out=rsum[:, :1],
            )
            rinv = work.tile([128, 1], F32, tag="rinv")
            nc.vector.reciprocal(rinv[:], rsum[:])
            probs = work.tile([128, S], F32, tag="probs")
            nc.vector.tensor_scalar_mul(probs[:], ex[:], rinv[:, :1])

            # ---- ids as f32 + replicated id row ----
            ids_f = work.tile([128, KS], F32, tag="ids_f")
            nc.vector.tensor_copy(ids_f[:], ids_sb[:])
            ids_row = work.tile([128, S], F32, tag="ids_row")
            for c in range(KS):
                pt = psum.tile([128, 128], F32, tag="tp")
                nc.tensor.transpose(
                    out=pt[:],
                    in_=ids_f[:, c:c + 1].to_broadcast([128, 128]),
                    identity=ident[:],
                )
                nc.vector.tensor_copy(ids_row[:, c * 128:(c + 1) * 128], pt[:])

            # ---- C_k[p, f] = (ids[128k+p] == ids[f])  (f32 compare, bf16 out) ----
            C = []
            for k in range(KS):
                ck = work.tile([128, S], F32, tag=f"C{k}")
                nc.vector.tensor_tensor(
                    out=ck[:],
                    in0=ids_f[:, k:k + 1].to_broadcast([128, S]),
                    in1=ids_row[:],
                    op=mybir.AluOpType.is_equal,
                )
                C.append(ck)

            # ---- probsT ----
            pT = []
            for k in range(KS):
                pt = psum.tile([128, 128], F32, tag="tp")
                nc.tensor.transpose(
                    out=pt[:], in_=probs[:, k * 128:(k + 1) * 128],
                    identity=ident[:],
                )
                t_ = work.tile([128, 128], F32, tag=f"pT{k}")
                nc.vector.tensor_copy(t_[:], pt[:])
                pT.append(t_)

            # ---- probs2T[j] = sum_k C_k[:, j]^T @ probsT_k  (group sums) ----
            p2T = []
            for j in range(KS):
                pj = psum.tile([128, 128], F32, tag="tp")
                for k in range(KS):
                    nc.tensor.matmul(
                        pj[:], lhsT=C[k][:, j * 128:(j + 1) * 128], rhs=pT[k][:],
                        start=(k == 0), stop=(k == KS - 1),
                    )
                t_ = work.tile([128, 128], STG_DT, tag=f"p2T{j}")
                nc.vector.tensor_copy(t_[:], pj[:])
                p2T.append(t_)

            # ---- indirect row scatter: stg[ids[s], :] = p2T rows ----
            for j in range(KS):
                nc.gpsimd.indirect_dma_start(
                    out=stg_ap,
                    out_offset=IndirectOffsetOnAxis(ap=ids_sb[:, j:j + 1], axis=0),
                    in_=p2T[j][:],
                    in_offset=None,
                )

            # Bridge the scatter window with dummy PE work so the HAM
            # clock gate stays at full rate into the transpose pass.
            for wu in range(64):
                wpt = psum16.tile(
                    [128, 128], STG_DT, tag="tp16", name=f"wb{wu}")
                nc.tensor.transpose(
                    out=wpt[:], in_=ident16[:], identity=ident16[:])

            # ---- blocked transpose pass: stg [V, TC] -> out [TC, V] ----
            for g in range(NG):
                vblk = tpp.tile([128, VG], STG_DT, tag="vblk")
                # vblk[p, c*128+f] = stg[g*VG + c*128 + p, f]
                vsrc = stg_ap[g * VG:(g + 1) * VG, :].rearrange(
                    "(c p) t -> p c t", p=128)
                nc.sync.dma_start(
                    vblk[:].rearrange("p (c t) -> p c t", c=NVB), vsrc)
                ostage = tpp.tile([128, VG], STG_DT, tag="ostage")
                for c in range(NVB):
                    pt = psum16.tile([128, 128], STG_DT, tag="tp16")
                    nc.tensor.transpose(
                        out=pt[:], in_=vblk[:, c * 128:(c + 1) * 128],
                        identity=ident16[:],
                    )
                    if c % 2 == 0:
                        nc.vector.tensor_copy(
                            ostage[:, c * 128:(c + 1) * 128], pt[:])
                    else:
                        nc.scalar.copy(ostage[:, c * 128:(c + 1) * 128], pt[:])
                nc.gpsimd.dma_start(out[:, g * VG:(g + 1) * VG], ostage[:])

    nc.finalize()
    return nc


_NC_CACHE = None


def _get_nc():
    global _NC_CACHE
    if _NC_CACHE is None:
        _NC_CACHE = build_bass()
    return _NC_CACHE


def kernel(**inputs: np.ndarray) -> np.ndarray:
    E = np.asarray(inputs["encoder_outputs"], dtype=np.float32)
    D = np.asarray(inputs["decoder_outputs"], dtype=np.float32)
    ids = np.ascontiguousarray(np.asarray(inputs["inputs"]).astype(np.int32))
    ib = np.ascontiguousarray(np.asarray(inputs["input_bias"], dtype=np.float32))
    W = np.ascontiguousarray(np.asarray(inputs["W_proj"]).astype(MM_NP))
    bp = np.ascontiguousarray(np.asarray(inputs["b_proj"], dtype=np.float32))

    nc = _get_nc()
    in_maps = []
    ETs = [np.ascontiguousarray(E[b].T.astype(MM_NP)) for b in range(B)]  # [H, S]
    for c in range(N_CORES):
        b, th = c // 2, c % 2
        in_maps.append({
            "eT": ETs[b],
            "dT": np.ascontiguousarray(
                D[b, th * TC:(th + 1) * TC].T.astype(MM_NP)),  # [H, TC]
            "w": W,
            "ids": ids[b],
            "sbias": ib[b],
            "bproj": bp,
        })
    res = run_bass_kernel_spmd(nc, in_maps, core_ids=list(range(N_CORES)))
    out = np.empty((B, T, V), dtype=np.float32)
    for c in range(N_CORES):
        b, th = c // 2, c % 2
        out[b, th * TC:(th + 1) * TC] = res.results[c]["out"]
    return out


if __name__ == "__main__":
    nc = build_bass()
    print("built ok")


# revision 16
# speedup vs baseline: 1.0165x; 1.0165x over previous
"""Trainium2 Bass kernel for nn_CopyStack (copy-mechanism vocab scatter).

Computes, for full inputs:
    enc   = tanh(encoder_outputs @ W_proj + b_proj)          [B,S,H]
    score = decoder_outputs @ enc^T + input_bias             [B,T,S]
    probs = softmax(score, axis=-1)                          [B,T,S]
    out[b,t,v] = sum_{s: inputs[b,s]==v} probs[b,t,s]        [B,T,V]

Sharding: 8 cores; core c handles batch b=c//2, target rows
t in [128*(c%2), 128*(c%2)+128). W_proj/b_proj replicated; E/D are
passed host-transposed ([H,S]/[H,TC]) and fp16-cast so matmul operands
load directly with no on-chip transposes (fp16 keeps an 11-bit
mantissa — measured end-to-end rel err ~4e-3 vs the 2e-2 gate).

Dataflow per core:
  - HWDGE zero-fills a [V, TC] bf16 staging buffer in DRAM (overlapped
    with compute).
  - encT = tanh(W^T E^T + b) via PE with fp16 operands (full-rate
    matmul + fast weight loads), k-outer accumulation into 4 PSUM
    banks so matmuls start as soon as the first chunks land.
  - softmax on ACT/DVE; duplicate token ids are pre-combined with a
    matmul against the S x S equality matrix C (every duplicate column
    carries its group sum, so colliding DMA writes are identical).
  - GPSIMD indirect DMA scatters bf16 rows of probs2^T into stg[V,TC].
  - 10 groups of 3200 v-rows are read back, PE-transposed (bf16,
    1 cyc/row), and streamed to the [TC, V] bf16 output; the host
    widens to f32 during the gather.
"""

import numpy as np

import concourse.bacc as bacc
import concourse.bass as bass
import concourse.tile as tile
from concourse import mybir
from concourse.bass import IndirectOffsetOnAxis
from concourse.bass_utils import run_bass_kernel_spmd
from concourse.masks import make_identity

F32 = mybir.dt.float32
F16 = mybir.dt.float16
BF16 = mybir.dt.bfloat16
I32 = mybir.dt.int32

B, S, T, H, V = 4, 512, 256, 1024, 32000
TC = 128             # T-chunk per core
N_CORES = 8

KH = H // 128        # 8 hidden chunks
KS = S // 128        # 4 source chunks
VG = 3200            # v-rows per readback group
NVB = VG // 128      # 25 v-blocks per group
NG = V // VG         # 10 groups

MM_DT = F16          # matmul operand dtype (host casts E/D/W)
STG_DT = BF16        # staging + output dtype (full f32 exponent range)
MM_NP = np.float16


def build_bass():
    nc = bacc.Bacc()

    eT_d = nc.dram_tensor("eT", [H, S], MM_DT, kind="ExternalInput")   # enc^T input
    dT_d = nc.dram_tensor("dT", [H, TC], MM_DT, kind="ExternalInput")  # dec chunk^T
    w = nc.dram_tensor("w", [H, H], MM_DT, kind="ExternalInput")       # W_proj
    ids = nc.dram_tensor("ids", [S], I32, kind="ExternalInput")        # inputs[b]
    sbias = nc.dram_tensor("sbias", [S], F32, kind="ExternalInput")    # input_bias[b]
    bproj = nc.dram_tensor("bproj", [H], F32, kind="ExternalInput")    # b_proj

    out = nc.dram_tensor("out", [TC, V], STG_DT, kind="ExternalOutput")

    with tile.TileContext(nc) as tc:
        with (
            tc.tile_pool(name="big", bufs=1) as big,
            tc.tile_pool(name="work", bufs=1) as work,
            tc.tile_pool(name="tpp", bufs=3) as tpp,
            tc.tile_pool(name="psum", bufs=2, space="PSUM") as psum,
            tc.tile_pool(name="psum16", bufs=2, space="PSUM") as psum16,
            tc.tile_pool(name="psumacc", bufs=4, space="PSUM") as psumacc,
            tc.tile_pool(name="dram", bufs=1, space="DRAM") as dram,
        ):
            stg = dram.tile([V, TC], STG_DT, tag="stg")
            stg_ap = stg[:, :]

            ident = work.tile([128, 128], F32, tag="ident")
            make_identity(nc, ident[:])
            ident16 = work.tile([128, 128], STG_DT, tag="ident16")
            nc.vector.tensor_copy(ident16[:], ident[:])

            # PE warm-up: sustained dummy work flips the HAM clock gate to
            # full rate (~3.4us of activity) before the real matmuls land.
            for wu in range(56):
                wpt = psum16.tile([128, 128], STG_DT, tag="tp16", name=f"wu{wu}")
                nc.tensor.transpose(
                    out=wpt[:], in_=ident16[:], identity=ident16[:])

            # ---- loads (no on-chip transposes: inputs come pre-transposed) ----
            w_t = []
            eT = []
            for k in range(KH):
                wt = big.tile([128, H], MM_DT, tag=f"w{k}")
                nc.sync.dma_start(wt[:], w[k * 128:(k + 1) * 128, :])
                w_t.append(wt)
                et = big.tile([128, S], MM_DT, tag=f"eT{k}")
                nc.sync.dma_start(et[:], eT_d[k * 128:(k + 1) * 128, :])
                eT.append(et)
            dT = []
            for k in range(KH):
                t_ = work.tile([128, TC], MM_DT, tag=f"dT{k}")
                nc.sync.dma_start(t_[:], dT_d[k * 128:(k + 1) * 128, :])
                dT.append(t_)

            ids_sb = work.tile([128, KS], I32, tag="ids")
            nc.sync.dma_start(ids_sb[:], ids[:].rearrange("(c p) -> p c", p=128))
            sbias_sb = work.tile([128, KS], F32, tag="sbias")
            nc.sync.dma_start(sbias_sb[:], sbias[:].rearrange("(c p) -> p c", p=128))
            bproj_sb = work.tile([128, KH], F32, tag="bproj")
            nc.sync.dma_start(bproj_sb[:], bproj[:].rearrange("(c p) -> p c", p=128))

            # ---- zero-fill the [V, TC] scatter buffer (big HWDGE DMAs) ----
            zt = big.tile([128, 4000], STG_DT, tag="zt")
            nc.vector.memset(zt[:], 0.0)
            stg_flat = stg_ap.rearrange("v t -> (v t)").rearrange(
                "(p f) -> p f", p=128)          # [128, 32000] flat view
            for j in range(8):
                nc.scalar.dma_start(stg_flat[:, j * 4000:(j + 1) * 4000], zt[:])

            # ---- encT[m] = tanh(W^T @ E^T + b) -> [128(h'), S] ----
            # k-outer so matmuls start when the first w/eT chunks land;
            # 4 PSUM banks per pass, 2 passes over m.
            encT = [None] * KH
            for half in range(2):
                ms = range(half * 4, half * 4 + 4)
                pms = {
                    m: psumacc.tile([128, S], F32, tag="mm", name=f"pm_{m}")
                    for m in ms
                }
                for k in range(KH):
                    for m in ms:
                        nc.tensor.matmul(
                            pms[m][:], lhsT=w_t[k][:, m * 128:(m + 1) * 128],
                            rhs=eT[k][:],
                            start=(k == 0), stop=(k == KH - 1),
                        )
                for m in ms:
                    et = big.tile([128, S], MM_DT, tag=f"encT{m}")
                    nc.scalar.activation(
                        et[:], pms[m][:], mybir.ActivationFunctionType.Tanh,
                        bias=bproj_sb[:, m:m + 1], scale=1.0,
                    )
                    encT[m] = et

            # ---- scores[t,s] = sum_h' dT[h',t] * encT[h',s] ----
            ps = psumacc.tile([128, S], F32, tag="mm")
            for k in range(KH):
                nc.tensor.matmul(
                    ps[:], lhsT=dT[k][:], rhs=encT[k][:],
                    start=(k == 0), stop=(k == KH - 1),
                )

            # bias row: [128, S] replicated over partitions, via PE transpose
            bias_row = work.tile([128, S], F32, tag="bias_row")
            for c in range(KS):
                pt = psum.tile([128, 128], F32, tag="tp")
                nc.tensor.transpose(
                    out=pt[:],
                    in_=sbias_sb[:, c:c + 1].to_broadcast([128, 128]),
                    identity=ident[:],
                )
                nc.vector.tensor_copy(bias_row[:, c * 128:(c + 1) * 128], pt[:])

            scoresb = work.tile([128, S], F32, tag="scoresb")
            nc.vector.tensor_tensor(
                out=scoresb[:], in0=ps[:], in1=bias_row[:], op=mybir.AluOpType.add,
            )

            # ---- softmax over s ----
            rmax = work.tile([128, 1], F32, tag="rmax")
            nc.vector.reduce_max(rmax[:], scoresb[:], axis=mybir.AxisListType.X)
            nrmax = work.tile([128, 1], F32, tag="nrmax")
            nc.vector.tensor_scalar_mul(nrmax[:], rmax[:], -1.0)
            ex = work.tile([128, S], F32, tag="ex")
            rsum = work.tile([128, 1], F32, tag="rsum")
            nc.scalar.activation(
                ex[:], scoresb[:], mybir.ActivationFunctionType.Exp,
                bias=nrmax[:, :1], scale=1.0, accum_out=rsum[:, :1],
            )
            rinv = work.tile([128, 1], F32, tag="rinv")
            nc.vector.reciprocal(rinv[:], rsum[:])
            probs = work.tile([128, S], F32, tag="probs")
            nc.vector.tensor_scalar_mul(probs[:], ex[:], rinv[:, :1])

            # ---- ids as f32 + replicated id row ----
            ids_f = work.tile([128, KS], F32, tag="ids_f")
            nc.vector.tensor_copy(ids_f[:], ids_sb[:])
            ids_row = work.tile([128, S], F32, tag="ids_row")
            for c in range(KS):
                pt = psum.tile([128, 128], F32, tag="tp")
                nc.tensor.transpose(
                    out=pt[:],
                    in_=ids_f[:, c:c + 1].to_broadcast([128, 128]),
                    identity=ident[:],
                )
                nc.vector.tensor_copy(ids_row[:, c * 128:(c + 1) * 128], pt[:])

            # ---- C_k[p, f] = (ids[128k+p] == ids[f])  (f32 compare, bf16 out) ----
            C = []
            for k in range(KS):
                ck = work.tile([128, S], F32, tag=f"C{k}")
                nc.vector.tensor_tensor(
                    out=ck[:],
                    in0=ids_f[:, k:k + 1].to_broadcast([128, S]),
                    in1=ids_row[:],
                    op=mybir.AluOpType.is_equal,
                )
                C.append(ck)

            # ---- probsT ----
            pT = []
            for k in range(KS):
                pt = psum.tile([128, 128], F32, tag="tp")
                nc.tensor.transpose(
                    out=pt[:], in_=probs[:, k * 128:(k + 1) * 128],
                    identity=ident[:],
                )
                t_ = work.tile([128, 128], F32, tag=f"pT{k}")
                nc.vector.tensor_copy(t_[:], pt[:])
                pT.append(t_)

            # ---- probs2T[j] = sum_k C_k[:, j]^T @ probsT_k  (group sums) ----
            p2T = []
            for j in range(KS):
                pj = psum.tile([128, 128], F32, tag="tp")
                for k in range(KS):
                    nc.tensor.matmul(
                        pj[:], lhsT=C[k][:, j * 128:(j + 1) * 128], rhs=pT[k][:],
                        start=(k == 0), stop=(k == KS - 1),
                    )
                t_ = work.tile([128, 128], STG_DT, tag=f"p2T{j}")
                nc.vector.tensor_copy(t_[:], pj[:])
                p2T.append(t_)

            # ---- indirect row scatter: stg[ids[s], :] = p2T rows ----
            for j in range(KS):
                nc.gpsimd.indirect_dma_start(
                    out=stg_ap,
                    out_offset=IndirectOffsetOnAxis(ap=ids_sb[:, j:j + 1], axis=0),
                    in_=p2T[j][:],
                    in_offset=None,
                )

            # ---- blocked transpose pass: stg [V, TC] -> out [TC, V] ----
            for g in range(NG):
                vblk = tpp.tile([128, VG], STG_DT, tag="vblk")
                # vblk[p, c*128+f] = stg[g*VG + c*128 + p, f]
                vsrc = stg_ap[g * VG:(g + 1) * VG, :].rearrange(
                    "(c p) t -> p c t", p=128)
                nc.sync.dma_start(
                    vblk[:].rearrange("p (c t) -> p c t", c=NVB), vsrc)
                ostage = tpp.tile([128, VG], STG_DT, tag="ostage")
                for c in range(NVB):
                    pt = psum16.tile([128, 128], STG_DT, tag="tp16")
                    nc.tensor.transpose(
                        out=pt[:], in_=vblk[:, c * 128:(c + 1) * 128],
                        identity=ident16[:],
                    )
                    if c % 2 == 0:
                        nc.vector.tensor_copy(
                            ostage[:, c * 128:(c + 1) * 128], pt[:])
                    else:
                        nc.scalar.copy(ostage[:, c * 128:(c + 1) * 128], pt[:])
                nc.gpsimd.dma_start(out[:, g * VG:(g + 1) * VG], ostage[:])

    nc.finalize()
    return nc


_NC_CACHE = None


def _get_nc():
    global _NC_CACHE
    if _NC_CACHE is None:
        _NC_CACHE = build_bass()
    return _NC_CACHE


def kernel(**inputs: np.ndarray) -> np.ndarray:
    E = np.asarray(inputs["encoder_outputs"], dtype=np.float32)
    D = np.asarray(inputs["decoder_outputs"], dtype=np.float32)
    ids = np.ascontiguousarray(np.asarray(inputs["inputs"]).astype(np.int32))
    ib = np.ascontiguousarray(np.asarray(inputs["input_bias"], dtype=np.float32))
    W = np.ascontiguousarray(np.asarray(inputs["W_proj"]).astype(MM_NP))
    bp = np.ascontiguousarray(np.asarray(inputs["b_proj"], dtype=np.float32))

    nc = _get_nc()
    in_maps = []
    ETs = [np.ascontiguousarray(E[b].T.astype(MM_NP)) for b in range(B)]  # [H, S]
    for c in range(N_CORES):
        b, th = c // 2, c % 2
        in_maps.append({
            "eT": ETs[b],
            "dT": np.ascontiguousarray(
                D[b, th * TC:(th + 1) * TC].T.astype(MM_NP)),  # [H, TC]
            "w": W,
            "ids": ids[b],
            "sbias": ib[b],
            "bproj": bp,
        })
    res = run_bass_kernel_spmd(nc, in_maps, core_ids=list(range(N_CORES)))
    out = np.empty((B, T, V), dtype=np.float32)
    for c in range(N_CORES):
        b, th = c // 2, c % 2
        out[b, th * TC:(th + 1) * TC] = res.results[c]["out"]
    return out


if __name__ == "__main__":
    nc = build_bass()
    print("built ok")


# revision 18
# speedup vs baseline: 1.1858x; 1.1666x over previous
"""Trainium2 Bass kernel for nn_CopyStack (copy-mechanism vocab scatter).

Computes, for full inputs:
    enc   = tanh(encoder_outputs @ W_proj + b_proj)          [B,S,H]
    score = decoder_outputs @ enc^T + input_bias             [B,T,S]
    probs = softmax(score, axis=-1)                          [B,T,S]
    out[b,t,v] = sum_{s: inputs[b,s]==v} probs[b,t,s]        [B,T,V]

Sharding: 8 cores; core c handles batch b=c//2, target rows
t in [128*(c%2), 128*(c%2)+128). W_proj/b_proj replicated; E/D are
passed host-transposed ([H,S]/[H,TC]) and fp16-cast so matmul operands
load directly with no on-chip transposes (fp16 keeps an 11-bit
mantissa — measured end-to-end rel err ~4e-3 vs the 2e-2 gate).

Dataflow per core:
  - HWDGE zero-fills a [V, TC] bf16 staging buffer in DRAM (overlapped
    with compute).
  - encT = tanh(W^T E^T + b) via PE with fp16 operands (full-rate
    matmul + fast weight loads), k-outer accumulation into 4 PSUM
    banks so matmuls start as soon as the first chunks land.
  - softmax on ACT/DVE; duplicate token ids are pre-combined with a
    matmul against the S x S equality matrix C (every duplicate column
    carries its group sum, so colliding DMA writes are identical).
  - GPSIMD indirect DMA scatters bf16 rows of probs2^T into stg[V,TC].
  - 10 groups of 3200 v-rows are read back, PE-transposed (bf16,
    1 cyc/row), and streamed to the [TC, V] bf16 output; the host
    widens to f32 during the gather.
"""

import numpy as np

import concourse.bacc as bacc
import concourse.bass as bass
import concourse.tile as tile
from concourse import mybir
from concourse.bass import IndirectOffsetOnAxis
from concourse.bass_utils import run_bass_kernel_spmd
from concourse.masks import make_identity

F32 = mybir.dt.float32
F16 = mybir.dt.float16
BF16 = mybir.dt.bfloat16
I32 = mybir.dt.int32

B, S, T, H, V = 4, 512, 256, 1024, 32000
TC = 128             # T-chunk per core
N_CORES = 8

KH = H // 128        # 8 hidden chunks
KS = S // 128        # 4 source chunks
VG = 3200            # v-rows per readback group
NVB = VG // 128      # 25 v-blocks per group
NG = V // VG         # 10 groups

MM_DT = F16          # matmul operand dtype (host casts E/D/W)
STG_DT = BF16        # staging + output dtype (full f32 exponent range)
MM_NP = np.float16


def build_bass():
    nc = bacc.Bacc()

    eT_d = nc.dram_tensor("eT", [H, S], MM_DT, kind="ExternalInput")   # enc^T input
    dT_d = nc.dram_tensor("dT", [H, TC], MM_DT, kind="ExternalInput")  # dec chunk^T
    w = nc.dram_tensor("w", [H, H], MM_DT, kind="ExternalInput")       # W_proj
    ids = nc.dram_tensor("ids", [S], I32, kind="ExternalInput")        # inputs[b]
    sbias = nc.dram_tensor("sbias", [S], F32, kind="ExternalInput")    # input_bias[b]
    bproj = nc.dram_tensor("bproj", [H], F32, kind="ExternalInput")    # b_proj

    out = nc.dram_tensor("out", [TC, V], STG_DT, kind="ExternalOutput")

    with tile.TileContext(nc) as tc:
        with (
            tc.tile_pool(name="big", bufs=1) as big,
            tc.tile_pool(name="work", bufs=1) as work,
            tc.tile_pool(name="tpp", bufs=4) as tpp,
            tc.tile_pool(name="psum", bufs=2, space="PSUM") as psum,
            tc.tile_pool(name="psum16", bufs=4, space="PSUM") as psum16,
            tc.tile_pool(name="psumacc", bufs=2, space="PSUM") as psumacc,
            tc.tile_pool(name="dram", bufs=1, space="DRAM") as dram,
        ):
            stg = dram.tile([V, TC], STG_DT, tag="stg")
            stg_ap = stg[:, :]

            ident = work.tile([128, 128], F32, tag="ident")
            make_identity(nc, ident[:])
            ident16 = work.tile([128, 128], STG_DT, tag="ident16")
            nc.vector.tensor_copy(ident16[:], ident[:])

            # PE warm-up: sustained dummy work flips the HAM clock gate to
            # full rate (~3.4us of activity) before the real matmuls land.
            for wu in range(56):
                wpt = psum16.tile([128, 128], STG_DT, tag="tp16", name=f"wu{wu}")
                nc.tensor.transpose(
                    out=wpt[:], in_=ident16[:], identity=ident16[:])

            # ---- loads (no on-chip transposes: inputs come pre-transposed) ----
            w_t = []
            eT = []
            for k in range(KH):
                wt = big.tile([128, H], MM_DT, tag=f"w{k}")
                nc.sync.dma_start(wt[:], w[k * 128:(k + 1) * 128, :])
                w_t.append(wt)
                et = big.tile([128, S], MM_DT, tag=f"eT{k}")
                nc.sync.dma_start(et[:], eT_d[k * 128:(k + 1) * 128, :])
                eT.append(et)
            dT = []
            for k in range(KH):
                t_ = work.tile([128, TC], MM_DT, tag=f"dT{k}")
                nc.sync.dma_start(t_[:], dT_d[k * 128:(k + 1) * 128, :])
                dT.append(t_)

            ids_sb = work.tile([128, KS], I32, tag="ids")
            nc.sync.dma_start(ids_sb[:], ids[:].rearrange("(c p) -> p c", p=128))
            sbias_sb = work.tile([128, KS], F32, tag="sbias")
            nc.sync.dma_start(sbias_sb[:], sbias[:].rearrange("(c p) -> p c", p=128))
            bproj_sb = work.tile([128, KH], F32, tag="bproj")
            nc.sync.dma_start(bproj_sb[:], bproj[:].rearrange("(c p) -> p c", p=128))

            # ---- zero-fill the [V, TC] scatter buffer (big HWDGE DMAs) ----
            zt = big.tile([128, 4000], STG_DT, tag="zt")
            nc.vector.memset(zt[:], 0.0)
            stg_flat = stg_ap.rearrange("v t -> (v t)").rearrange(
                "(p f) -> p f", p=128)          # [128, 32000] flat view
            for j in range(8):
                nc.scalar.dma_start(stg_flat[:, j * 4000:(j + 1) * 4000], zt[:])

            # ---- encT[m] = tanh(W^T @ E^T + b) -> [128(h'), S] ----
            # k-outer so matmuls start when the first w/eT chunks land;
            # 4 PSUM banks per pass, 2 passes over m.
            encT = [None] * KH
            for half in range(4):
                ms = range(half * 2, half * 2 + 2)
                pms = {
                    m: psumacc.tile([128, S], F32, tag="mm", name=f"pm_{m}")
                    for m in ms
                }
                for k in range(KH):
                    for m in ms:
                        nc.tensor.matmul(
                            pms[m][:], lhsT=w_t[k][:, m * 128:(m + 1) * 128],
                            rhs=eT[k][:],
                            start=(k == 0), stop=(k == KH - 1),
                        )
                for m in ms:
                    et = big.tile([128, S], MM_DT, tag=f"encT{m}")
                    nc.scalar.activation(
                        et[:], pms[m][:], mybir.ActivationFunctionType.Tanh,
                        bias=bproj_sb[:, m:m + 1], scale=1.0,
                    )
                    encT[m] = et

            # ---- scores[t,s] = sum_h' dT[h',t] * encT[h',s] ----
            ps = psumacc.tile([128, S], F32, tag="mm")
            for k in range(KH):
                nc.tensor.matmul(
                    ps[:], lhsT=dT[k][:], rhs=encT[k][:],
                    start=(k == 0), stop=(k == KH - 1),
                )

            # bias row: [128, S] replicated over partitions, via PE transpose
            bias_row = work.tile([128, S], F32, tag="bias_row")
            for c in range(KS):
                pt = psum.tile([128, 128], F32, tag="tp")
                nc.tensor.transpose(
                    out=pt[:],
                    in_=sbias_sb[:, c:c + 1].to_broadcast([128, 128]),
                    identity=ident[:],
                )
                nc.vector.tensor_copy(bias_row[:, c * 128:(c + 1) * 128], pt[:])

            scoresb = work.tile([128, S], F32, tag="scoresb")
            nc.vector.tensor_tensor(
                out=scoresb[:], in0=ps[:], in1=bias_row[:], op=mybir.AluOpType.add,
            )

            # ---- softmax over s ----
            rmax = work.tile([128, 1], F32, tag="rmax")
            nc.vector.reduce_max(rmax[:], scoresb[:], axis=mybir.AxisListType.X)
            nrmax = work.tile([128, 1], F32, tag="nrmax")
            nc.vector.tensor_scalar_mul(nrmax[:], rmax[:], -1.0)
            ex = work.tile([128, S], F32, tag="ex")
            rsum = work.tile([128, 1], F32, tag="rsum")
            nc.scalar.activation(
                ex[:], scoresb[:], mybir.ActivationFunctionType.Exp,
                bias=nrmax[:, :1], scale=1.0, accum_out=rsum[:, :1],
            )
            rinv = work.tile([128, 1], F32, tag="rinv")
            nc.vector.reciprocal(rinv[:], rsum[:])
            probs = work.tile([128, S], F32, tag="probs")
            nc.vector.tensor_scalar_mul(probs[:], ex[:], rinv[:, :1])

            # ---- ids as f32 + replicated id row ----
            ids_f = work.tile([128, KS], F32, tag="ids_f")
            nc.vector.tensor_copy(ids_f[:], ids_sb[:])
            ids_row = work.tile([128, S], F32, tag="ids_row")
            for c in range(KS):
                pt = psum.tile([128, 128], F32, tag="tp")
                nc.tensor.transpose(
                    out=pt[:],
                    in_=ids_f[:, c:c + 1].to_broadcast([128, 128]),
                    identity=ident[:],
                )
                nc.vector.tensor_copy(ids_row[:, c * 128:(c + 1) * 128], pt[:])

            # ---- C_k[p, f] = (ids[128k+p] == ids[f])  (f32 compare, bf16 out) ----
            C = []
            for k in range(KS):
                ck = work.tile([128, S], F32, tag=f"C{k}")
                nc.vector.tensor_tensor(
                    out=ck[:],
                    in0=ids_f[:, k:k + 1].to_broadcast([128, S]),
                    in1=ids_row[:],
                    op=mybir.AluOpType.is_equal,
                )
                C.append(ck)

            # ---- probsT ----
            pT = []
            for k in range(KS):
                pt = psum.tile([128, 128], F32, tag="tp")
                nc.tensor.transpose(
                    out=pt[:], in_=probs[:, k * 128:(k + 1) * 128],
                    identity=ident[:],
                )
                t_ = work.tile([128, 128], F32, tag=f"pT{k}")
                nc.vector.tensor_copy(t_[:], pt[:])
                pT.append(t_)

            # ---- probs2T[j] = sum_k C_k[:, j]^T @ probsT_k  (group sums) ----
            p2T = []
            for j in range(KS):
                pj = psum.tile([128, 128], F32, tag="tp")
                for k in range(KS):
                    nc.tensor.matmul(
                        pj[:], lhsT=C[k][:, j * 128:(j + 1) * 128], rhs=pT[k][:],
                        start=(k == 0), stop=(k == KS - 1),
                    )
                t_ = work.tile([128, 128], STG_DT, tag=f"p2T{j}")
                nc.vector.tensor_copy(t_[:], pj[:])
                p2T.append(t_)

            # ---- indirect row scatter: stg[ids[s], :] = p2T rows ----
            for j in range(KS):
                nc.gpsimd.indirect_dma_start(
                    out=stg_ap,
                    out_offset=IndirectOffsetOnAxis(ap=ids_sb[:, j:j + 1], axis=0),
                    in_=p2T[j][:],
                    in_offset=None,
                )

            # ---- blocked transpose pass: stg [V, TC] -> out [TC, V] ----
            for g in range(NG):
                vblk = tpp.tile([128, VG], STG_DT, tag="vblk")
                # vblk[p, c*128+f] = stg[g*VG + c*128 + p, f]
                vsrc = stg_ap[g * VG:(g + 1) * VG, :].rearrange(
                    "(c p) t -> p c t", p=128)
                nc.sync.dma_start(
                    vblk[:].rearrange("p (c t) -> p c t", c=NVB), vsrc)
                ostage = tpp.tile([128, VG], STG_DT, tag="ostage")
                for c in range(NVB):
                    pt = psum16.tile(
                        [128, 128], STG_DT, tag="tp16", name=f"pc{g}_{c}")
                    nc.tensor.transpose(
                        out=pt[:], in_=vblk[:, c * 128:(c + 1) * 128],
                        identity=ident16[:],
                    )
                    if c % 2 == 0:
                        nc.vector.tensor_copy(
                            ostage[:, c * 128:(c + 1) * 128], pt[:])
                    else:
                        nc.scalar.copy(ostage[:, c * 128:(c + 1) * 128], pt[:])
                nc.gpsimd.dma_start(out[:, g * VG:(g + 1) * VG], ostage[:])

    nc.finalize()
    return nc


_NC_CACHE = None


def _get_nc():
    global _NC_CACHE
    if _NC_CACHE is None:
        _NC_CACHE = build_bass()
    return _NC_CACHE


def kernel(**inputs: np.ndarray) -> np.ndarray:
    E = np.asarray(inputs["encoder_outputs"], dtype=np.float32)
    D = np.asarray(inputs["decoder_outputs"], dtype=np.float32)
    ids = np.ascontiguousarray(np.asarray(inputs["inputs"]).astype(np.int32))
    ib = np.ascontiguousarray(np.asarray(inputs["input_bias"], dtype=np.float32))
    W = np.ascontiguousarray(np.asarray(inputs["W_proj"]).astype(MM_NP))
    bp = np.ascontiguousarray(np.asarray(inputs["b_proj"], dtype=np.float32))

    nc = _get_nc()
    in_maps = []
    ETs = [np.ascontiguousarray(E[b].T.astype(MM_NP)) for b in range(B)]  # [H, S]
    for c in range(N_CORES):
        b, th = c // 2, c % 2
        in_maps.append({
            "eT": ETs[b],
            "dT": np.ascontiguousarray(
                D[b, th * TC:(th + 1) * TC].T.astype(MM_NP)),  # [H, TC]
            "w": W,
            "ids": ids[b],
            "sbias": ib[b],
            "bproj": bp,
        })
    res = run_bass_kernel_spmd(nc, in_maps, core_ids=list(range(N_CORES)))
    out = np.empty((B, T, V), dtype=np.float32)
    for c in range(N_CORES):
        b, th = c // 2, c % 2
        out[b, th * TC:(th + 1) * TC] = res.results[c]["out"]
    return out


if __name__ == "__main__":
    nc = build_bass()
    print("built ok")
